# revision 1
# baseline (speedup 1.0000x reference)
# Self-contained TRN2 Bass kernel for nn_Attention_NMT (B=32,S=T=100,H=E=512,V=32000).
# SPMD over 8 NeuronCores, batch-parallel (4 batch rows per core).
import sys
for _p in ("/opt/trn_rl_repo",):
    if _p not in sys.path:
        sys.path.insert(0, _p)
import numpy as np
import ml_dtypes
BF = ml_dtypes.bfloat16
import concourse.bass as bass
import concourse.bacc as bacc
import concourse.tile as tile
from concourse import mybir
from concourse.bass import ds
from concourse.masks import make_identity

F32 = mybir.dt.float32
F32R = mybir.dt.float32r
BF16 = mybir.dt.bfloat16
I32 = mybir.dt.int32
AF = mybir.ActivationFunctionType
ALU = mybir.AluOpType

E = 512; H = 512; G = 2048; A = 1536; Bl = 4
EK = E // 128; HK = H // 128; H2K = 2 * H // 128; AJ = A // 128
GN = G // 512; AN = A // 512


def r(ap):
    return ap.bitcast(F32R)


def build_nc(S=100, T=100, V=32000, num_devices=8, unroll=4):
    NQ = (S + 31) // 32          # source quarters / token tiles
    NQT = (T + 31) // 32
    SP = 32 * NQ                 # padded
    TP = 32 * NQT

    nc = bacc.Bacc("TRN2", target_bir_lowering=False, debug=False,
                   num_devices=num_devices)

    def din(name, shape, dt=F32):
        return nc.dram_tensor(name, shape, dt, kind="ExternalInput")

    src_emb = din("src_emb", [V, E]); tgt_emb = din("tgt_emb", [V, E])
    idx_src = din("idx_src", [128, NQ], I32)
    idx_tgt = din("idx_tgt", [128, NQT], I32)
    WihfT = din("WihfT", [E, G], BF16); WihbT = din("WihbT", [E, G], BF16)
    WiheT = din("WiheT", [E, G], BF16)
    WhhfT = din("WhhfT", [H, G], BF16); WhhbT = din("WhhbT", [H, G], BF16)
    WqWhhT = din("WqWhhT", [H, A + G], BF16)
    WihcT = din("WihcT", [2 * H, G], BF16)
    A1eT = din("A1eT", [2 * H, A], BF16)
    a2T_in = din("a2T", [128, AJ])
    biasf = din("biasf", [1, G], BF16); biasb = din("biasb", [1, G], BF16)
    biasd = din("biasd", [1, G], BF16); att1b = din("att1b", [1, A], BF16)
    cls1WT = din("cls1WT", [G, 1024], BF16); cls1b = din("cls1b", [1, 1024], BF16)
    cls2WT = din("cls2WT", [1024, V], BF16); cls2b = din("cls2b", [1, V], BF16)
    mask4 = din("mask4", [128, Bl])

    logits = nc.dram_tensor("logits", [T, Bl, V], F32, kind="ExternalOutput")

    # DRAM scratch
    Xf_d = nc.dram_tensor("Xf_d", [S, Bl, G], BF16)
    Xb_d = nc.dram_tensor("Xb_d", [S, Bl, G], BF16)
    Xd_d = nc.dram_tensor("Xd_d", [T, Bl, G], BF16)
    of_d = nc.dram_tensor("of_d", [S, Bl, H], BF16)
    ob_d = nc.dram_tensor("ob_d", [S, Bl, H], BF16)
    ctx_d = nc.dram_tensor("ctx_d", [T, Bl, 2 * H], BF16)
    dec_d = nc.dram_tensor("dec_d", [T, Bl, H], BF16)

    with tile.TileContext(nc) as tc:
        from contextlib import ExitStack
        _stack = ExitStack()
        persist = _stack.enter_context(tc.tile_pool(name="persist", bufs=1))

        # ---- constants ----
        I128 = persist.tile([128, 128], F32)
        make_identity(nc, I128[:, :])
        ones = persist.tile([1, 512], BF16)
        nc.vector.memset(ones[:, :], 1.0)
        I128b = persist.tile([128, 128], BF16)
        nc.vector.tensor_copy(I128b[:, :], I128[:, :])
        mask4_s = persist.tile([128, Bl], F32)
        nc.gpsimd.dma_start(out=mask4_s[:, :], in_=mask4[:, :])
        I4b = persist.tile([Bl, Bl], BF16)
        a2Tb = persist.tile([128, AJ], BF16)
        a2Tf = persist.tile([128, AJ], F32)
        nc.gpsimd.dma_start(out=a2Tf[:, :], in_=a2T_in[:, :])
        nc.vector.tensor_copy(a2Tb[:, :], a2Tf[:, :])
        nc.vector.tensor_copy(I4b[:, :], I128[:Bl, :Bl])
        att1b_s = persist.tile([1, A], BF16)
        nc.gpsimd.dma_start(out=att1b_s[:, :], in_=att1b[:, :])

        # persistent activations
        tembT = persist.tile([128, EK, 128 * NQT], BF16)
        es = persist.tile([1, 512], F32)
        nc.vector.memset(es[:, :], 0.0)
        h_f = persist.tile([Bl, H], F32); c_f = persist.tile([Bl, H], F32)
        h_b = persist.tile([Bl, H], F32); c_b = persist.tile([Bl, H], F32)
        for t_ in (h_f, c_f, h_b, c_b):
            nc.vector.memset(t_[:, :], 0.0)

        # ================= PHASE 0: embeddings + X GEMMs =================
        with tc.tile_pool(name="ph0", bufs=1) as ph0, \
             tc.tile_pool(name="ph0ps", bufs=2, space="PSUM") as ph0ps, \
             tc.tile_pool(name="ph0st", bufs=2) as ph0st:
            idxs = ph0.tile([128, NQ], I32)
            nc.gpsimd.dma_start(out=idxs[:, :], in_=idx_src[:, :])
            idxt = ph0.tile([128, NQT], I32)
            nc.gpsimd.dma_start(out=idxt[:, :], in_=idx_tgt[:, :])
            xQ = ph0.tile([128, NQ, E], F32)
            tembQ = ph0.tile([128, NQT, E], F32)
            for q in range(NQ):
                nc.gpsimd.indirect_dma_start(
                    out=xQ[:, q, :], out_offset=None, in_=src_emb[:, :],
                    in_offset=bass.IndirectOffsetOnAxis(ap=idxs[:, q:q + 1], axis=0))
            for q in range(NQT):
                nc.gpsimd.indirect_dma_start(
                    out=tembQ[:, q, :], out_offset=None, in_=tgt_emb[:, :],
                    in_offset=bass.IndirectOffsetOnAxis(ap=idxt[:, q:q + 1], axis=0))

            # transpose xQ/tembQ -> xT/tembT  (feature-major, token cols)
            xT = ph0.tile([128, EK, 128 * NQ], BF16)
            for q in range(NQ):
                pT = ph0ps.tile([128, EK, 128], F32, space="PSUM")
                for kc in range(EK):
                    nc.tensor.transpose(out=pT[:, kc, :],
                                        in_=xQ[:, q, 128 * kc:128 * (kc + 1)],
                                        identity=I128[:, :])
                nc.vector.tensor_copy(xT[:, :, 128 * q:128 * (q + 1)], pT[:, :, :])
            for q in range(NQT):
                pT = ph0ps.tile([128, EK, 128], F32, space="PSUM")
                for kc in range(EK):
                    nc.tensor.transpose(out=pT[:, kc, :],
                                        in_=tembQ[:, q, 128 * kc:128 * (kc + 1)],
                                        identity=I128[:, :])
                nc.vector.tensor_copy(tembT[:, :, 128 * q:128 * (q + 1)], pT[:, :, :])

            # X GEMMs -> DRAM   (token-stationary, stream W)
            def x_gemm(wT_dram, bias_dram, lhsT_tile, nQ, S_, out_dram):
                Ws = ph0.tile([128, EK, G], BF16, tag="wstream")
                nc.gpsimd.dma_start(
                    out=Ws[:, :, :],
                    in_=wT_dram[:, :].rearrange("(k p) g -> p k g", p=128))
                bia = ph0.tile([1, G], BF16, tag="bias")
                nc.gpsimd.dma_start(out=bia[:, :], in_=bias_dram[:, :])
                for Tt in range(nQ):
                    rows = min(32, S_ - 32 * Tt) * Bl
                    stage = ph0st.tile([128, G], BF16, tag="xstage")
                    for n in range(GN):
                        ps = ph0ps.tile([128, 512], F32, space="PSUM", tag="xps")
                        for kc in range(EK):
                            nc.tensor.matmul(
                                out=ps[:, :],
                                lhsT=(lhsT_tile[:, kc, 128 * Tt:128 * (Tt + 1)]),
                                rhs=(Ws[:, kc, 512 * n:512 * (n + 1)]),
                                start=(kc == 0), stop=False)
                        nc.tensor.matmul(
                            out=ps[:, :], lhsT=(ones[:1, :128]),
                            rhs=(bia[:, 512 * n:512 * (n + 1)]),
                            start=False, stop=True)
                        nc.scalar.copy(stage[:, 512 * n:512 * (n + 1)], ps[:, :])
                    nc.gpsimd.dma_start(
                        out=out_dram[32 * Tt:32 * Tt + rows // Bl, :,
                                     :].flatten_outer_dims(),
                        in_=stage[:rows, :])
            x_gemm(WihfT, biasf, xT, NQ, S, Xf_d)
            x_gemm(WihbT, biasb, xT, NQ, S, Xb_d)
            x_gemm(WiheT, biasd, tembT, NQT, T, Xd_d)

        # ================= PHASE 1: encoder =================
        with tc.tile_pool(name="ph1w", bufs=1) as ph1w, \
             tc.tile_pool(name="xst", bufs=2) as xstp, \
             tc.tile_pool(name="cell", bufs=1) as cellp, \
             tc.tile_pool(name="hT", bufs=2) as hTp, \
             tc.tile_pool(name="encg", bufs=1, space="PSUM") as encg:
            Whhf_s = ph1w.tile([128, HK, G], BF16)
            nc.gpsimd.dma_start(out=Whhf_s[:, :, :],
                in_=WhhfT[:, :].rearrange("(k p) g -> p k g", p=128))
            Whhb_s = ph1w.tile([128, HK, G], BF16)
            nc.gpsimd.dma_start(out=Whhb_s[:, :, :],
                in_=WhhbT[:, :].rearrange("(k p) g -> p k g", p=128))

            def lstm_step(h, c, Whh_s, X_d, s_expr, store_d, tagp):
                xst = xstp.tile([Bl, G], BF16, tag="xst" + tagp)
                nc.gpsimd.dma_start(
                    out=xst[:, :], in_=X_d[ds(s_expr, 1)].flatten_outer_dims())
                gfull = encg.tile([128, G], F32, space="PSUM", tag="g" + tagp)
                for kc in range(HK):
                    nc.tensor.transpose(out=gfull[:, Bl * kc:Bl * (kc + 1)],
                                        in_=h[:, 128 * kc:128 * (kc + 1)],
                                        identity=I128[:Bl, :Bl])
                hTs = hTp.tile([128, HK, Bl], BF16, tag="hT" + tagp)
                nc.vector.tensor_copy(
                    hTs[:, :, :],
                    gfull[:, 0:HK * Bl].rearrange("p (k b) -> p k b", b=Bl))
                gates = gfull[0:Bl, :]
                for n in range(GN):
                    gsl = gates[:, 512 * n:512 * (n + 1)]
                    for kc in range(HK):
                        nc.tensor.matmul(out=gsl, lhsT=hTs[:, kc, :],
                                         rhs=Whh_s[:, kc, 512 * n:512 * (n + 1)],
                                         start=(kc == 0), stop=False)
                    nc.tensor.matmul(out=gsl, lhsT=I4b[:, :],
                                     rhs=xst[:, 512 * n:512 * (n + 1)],
                                     start=False, stop=True)
                sif = cellp.tile([Bl, 1024], F32, tag="sif" + tagp)
                nc.scalar.activation(out=sif[:, :], in_=gates[:, 0:1024],
                                     func=AF.Sigmoid)
                tg = cellp.tile([Bl, H], F32, tag="tg" + tagp)
                nc.scalar.activation(out=tg[:, :], in_=gates[:, 1024:1536],
                                     func=AF.Tanh)
                so = cellp.tile([Bl, H], F32, tag="so" + tagp)
                nc.scalar.activation(out=so[:, :], in_=gates[:, 1536:2048],
                                     func=AF.Sigmoid)
                p1 = cellp.tile([Bl, H], F32, tag="p1" + tagp)
                nc.vector.tensor_mul(p1[:, :], sif[:, 0:512], tg[:, :])
                p2 = cellp.tile([Bl, H], F32, tag="p2" + tagp)
                nc.vector.tensor_mul(p2[:, :], sif[:, 512:1024], c[:, :])
                nc.vector.tensor_add(c[:, :], p1[:, :], p2[:, :])
                tcn = cellp.tile([Bl, H], F32, tag="tc" + tagp)
                nc.scalar.activation(out=tcn[:, :], in_=c[:, :], func=AF.Tanh)
                nc.vector.tensor_mul(h[:, :], so[:, :], tcn[:, :])
                if store_d is not None:
                    hbf = cellp.tile([Bl, H], BF16, tag="hbf" + tagp)
                    nc.vector.tensor_copy(hbf[:, :], h[:, :])
                    nc.gpsimd.dma_start(
                        out=store_d[ds(s_expr, 1)].flatten_outer_dims(),
                        in_=hbf[:, :])

            with tc.For_i(0, S, unroll) as i0:
                for u_ in range(unroll):
                    lstm_step(h_f, c_f, Whhf_s, Xf_d, i0 + u_, of_d, "f")
                    lstm_step(h_b, c_b, Whhb_s, Xb_d, (S - 1 - u_) - i0, ob_d, "b")

        # ============ PHASE 2: assemble enc tiles + transposes ============
        scopeB_cm = tc.tile_pool(name="scopeB", bufs=1)
        scopeB = scopeB_cm.__enter__()
        ofQ = scopeB.tile([128, NQ, H], BF16)
        obQ = scopeB.tile([128, NQ, H], BF16)
        u_s = scopeB.tile([128, AJ, SP, Bl], BF16)
        encWcQ = scopeB.tile([128, NQ, G], BF16)
        ph23_cm = tc.tile_pool(name="ph23", bufs=1)
        ph23 = ph23_cm.__enter__()
        encT = ph23.tile([128, H2K, 128 * NQ], BF16)
        with tc.tile_pool(name="ph2ps", bufs=2, space="PSUM") as ph2ps:
            nc.vector.memset(ofQ[:, :, :], 0.0)
            nc.vector.memset(obQ[:, :, :], 0.0)
            for q in range(NQ):
                rows = min(32, S - 32 * q) * Bl
                nc.gpsimd.dma_start(
                    out=ofQ[:rows, q, :],
                    in_=of_d[32 * q:32 * q + rows // Bl].flatten_outer_dims())
                nc.gpsimd.dma_start(
                    out=obQ[:rows, q, :],
                    in_=ob_d[32 * q:32 * q + rows // Bl].flatten_outer_dims())
            for q in range(NQ):
                pT = ph2ps.tile([128, HK, 128], BF16, space="PSUM")
                for kc in range(HK):
                    nc.tensor.transpose(out=pT[:, kc, :],
                                        in_=ofQ[:, q, 128 * kc:128 * (kc + 1)],
                                        identity=I128b[:, :])
                nc.vector.tensor_copy(encT[:, 0:HK, 128 * q:128 * (q + 1)],
                                      pT[:, :, :])
                pT2 = ph2ps.tile([128, HK, 128], BF16, space="PSUM")
                for kc in range(HK):
                    nc.tensor.transpose(out=pT2[:, kc, :],
                                        in_=obQ[:, q, 128 * kc:128 * (kc + 1)],
                                        identity=I128b[:, :])
                nc.vector.tensor_copy(encT[:, HK:H2K, 128 * q:128 * (q + 1)],
                                      pT2[:, :, :])

        # ============ PHASE 3: u GEMM + encWcQ GEMM ============
        NTOK = 128 * NQ
        with tc.tile_pool(name="ph3", bufs=1) as ph3, \
             tc.tile_pool(name="ph3st", bufs=3) as ph3st, \
             tc.tile_pool(name="ph3ps", bufs=2, space="PSUM") as ph3ps:
            A1e_s = ph3.tile([128, H2K, A], BF16)
            nc.gpsimd.dma_start(out=A1e_s[:, :, :],
                in_=A1eT[:, :].rearrange("(k p) a -> p k a", p=128))
            for m in range(AJ):
                ps = ph3ps.tile([128, 512], F32, space="PSUM", tag="ups")
                for kc in range(H2K):
                    nc.tensor.matmul(out=ps[:, :NTOK],
                                     lhsT=(A1e_s[:, kc, 128 * m:128 * (m + 1)]),
                                     rhs=(encT[:, kc, :]),
                                     start=(kc == 0), stop=False)
                nc.tensor.matmul(out=ps[:, :NTOK], lhsT=(att1b_s[:1, 128 * m:128 * (m + 1)]),
                                 rhs=(ones[:1, :NTOK]), start=False, stop=True)
                nc.vector.tensor_copy(
                    u_s[:, m, :, :],
                    ps[:, :NTOK].rearrange("p (s b) -> p s b", b=Bl))
            # encWcQ: token-stationary, stream WihcT chunks from DRAM
            for n in range(GN):
                Wc_n = ph3st.tile([128, H2K, 512], BF16, tag="wcn")
                nc.gpsimd.dma_start(out=Wc_n[:, :, :],
                    in_=WihcT[:, 512 * n:512 * (n + 1)].rearrange(
                        "(k p) g -> p k g", p=128))
                for Tt in range(NQ):
                    ps = ph3ps.tile([128, 512], F32, space="PSUM", tag="wcps")
                    for kc in range(H2K):
                        nc.tensor.matmul(
                            out=ps[:, :],
                            lhsT=(encT[:, kc, 128 * Tt:128 * (Tt + 1)]),
                            rhs=(Wc_n[:, kc, :]), start=(kc == 0),
                            stop=(kc == H2K - 1))
                    nc.scalar.copy(encWcQ[:, Tt, 512 * n:512 * (n + 1)], ps[:, :])

        ph23_cm.__exit__(None, None, None)

        # ================= PHASE 4: decoder =================
        h = h_f; c = c_f   # decoder init = final fwd encoder state
        with tc.tile_pool(name="ph4w", bufs=1) as ph4w, \
             tc.tile_pool(name="dxst", bufs=2) as dxstp, \
             tc.tile_pool(name="dcell", bufs=1) as dcellp, \
             tc.tile_pool(name="dsb", bufs=2) as dsb, \
             tc.tile_pool(name="dps", bufs=1, space="PSUM") as dps:
            rl_s = ph4w.tile([128, AJ, SP, Bl], BF16)
            Wq_s = ph4w.tile([128, HK, A + G], BF16)
            nc.gpsimd.dma_start(out=Wq_s[:, :, :],
                in_=WqWhhT[:, :].rearrange("(k p) g -> p k g", p=128))

            def cell_update(gates, hh, cc, pool, tagp):
                sif = pool.tile([Bl, 1024], F32, tag="sif" + tagp)
                nc.scalar.activation(out=sif[:, :], in_=gates[:, 0:1024],
                                     func=AF.Sigmoid)
                tg = pool.tile([Bl, H], F32, tag="tg" + tagp)
                nc.scalar.activation(out=tg[:, :], in_=gates[:, 1024:1536],
                                     func=AF.Tanh)
                so = pool.tile([Bl, H], F32, tag="so" + tagp)
                nc.scalar.activation(out=so[:, :], in_=gates[:, 1536:2048],
                                     func=AF.Sigmoid)
                p1 = pool.tile([Bl, H], F32, tag="p1" + tagp)
                nc.vector.tensor_mul(p1[:, :], sif[:, 0:512], tg[:, :])
                p2 = pool.tile([Bl, H], F32, tag="p2" + tagp)
                nc.vector.tensor_mul(p2[:, :], sif[:, 512:1024], cc[:, :])
                nc.vector.tensor_add(cc[:, :], p1[:, :], p2[:, :])
                tcn = pool.tile([Bl, H], F32, tag="tc" + tagp)
                nc.scalar.activation(out=tcn[:, :], in_=cc[:, :], func=AF.Tanh)
                nc.vector.tensor_mul(hh[:, :], so[:, :], tcn[:, :])

            def dec_step(t_expr):
                xst = dxstp.tile([Bl, G], BF16, tag="dx")
                nc.gpsimd.dma_start(
                    out=xst[:, :], in_=Xd_d[ds(t_expr, 1)].flatten_outer_dims())
                # h transposes into gates-psum corner
                gfull = dps.tile([128, G], F32, space="PSUM", tag="dg")
                for kc in range(HK):
                    nc.tensor.transpose(out=gfull[:, Bl * kc:Bl * (kc + 1)],
                                        in_=h[:, 128 * kc:128 * (kc + 1)],
                                        identity=I128[:Bl, :Bl])
                hTs = dsb.tile([128, HK, Bl], BF16, tag="dhT")
                nc.vector.tensor_copy(
                    hTs[:, :, :],
                    gfull[:, 0:HK * Bl].rearrange("p (k b) -> p k b", b=Bl))
                # q proj + h-part of gates
                qfull = dps.tile([128, A], F32, space="PSUM", tag="qctx")
                qps = qfull[0:Bl, :]
                for n in range(AN):
                    for kc in range(HK):
                        nc.tensor.matmul(
                            out=qps[:, 512 * n:512 * (n + 1)],
                            lhsT=hTs[:, kc, :],
                            rhs=Wq_s[:, kc, 512 * n:512 * (n + 1)],
                            start=(kc == 0), stop=(kc == HK - 1))
                gates = gfull[0:Bl, :]
                for n in range(GN):
                    gsl = gates[:, 512 * n:512 * (n + 1)]
                    for kc in range(HK):
                        nc.tensor.matmul(
                            out=gsl, lhsT=hTs[:, kc, :],
                            rhs=Wq_s[:, kc, A + 512 * n:A + 512 * (n + 1)],
                            start=(kc == 0), stop=False)
                    nc.tensor.matmul(out=gsl, lhsT=I4b[:, :],
                                     rhs=xst[:, 512 * n:512 * (n + 1)],
                                     start=False, stop=False)
                qs = dsb.tile([Bl, A], F32, tag="dq")
                nc.scalar.copy(qs[:, :], qps[:, :])
                for j in range(AJ):
                    nc.tensor.transpose(
                        out=qfull[:, Bl * j:Bl * (j + 1)],
                        in_=qs[:, 128 * j:128 * (j + 1)],
                        identity=I128[:Bl, :Bl])
                qTs = dsb.tile([128, AJ, Bl], BF16, tag="dqT")
                nc.vector.tensor_copy(
                    qTs[:, :, :],
                    qfull[:, 0:AJ * Bl].rearrange("p (j b) -> p j b", b=Bl))
                # attention: rl = relu(u + q)
                qb = qTs[:, :, :]
                qbc = bass.AP(tensor=qb.tensor, offset=qb.offset,
                              ap=[qb.ap[0], qb.ap[1], [0, SP], qb.ap[2]])
                nc.vector.tensor_tensor(out=rl_s[:, :, :, :], in0=u_s[:, :, :, :],
                                        in1=qbc, op=ALU.add)
                nc.vector.tensor_scalar_max(rl_s[:, :, :, :], rl_s[:, :, :, :], 0.0)
                pscf = dps.tile([128, 512], F32, space="PSUM", tag="psc")
                psc = pscf[0:1, :]
                for j in range(AJ):
                    nc.tensor.matmul(out=psc[0:1, 0:SP * Bl],
                                     lhsT=a2Tb[:, j:j + 1],
                                     rhs=rl_s[:, j, :, :],
                                     start=(j == 0), stop=(j == AJ - 1))
                nc.scalar.activation(out=es[0:1, 0:S * Bl], in_=psc[0:1, 0:S * Bl],
                                     func=AF.Exp)
                ssum = dsb.tile([1, Bl], F32, tag="ssum")
                nc.vector.tensor_reduce(
                    out=ssum[:, :],
                    in_=es[0:1, 0:S * Bl].rearrange("p (s b) -> p b s", b=Bl),
                    axis=mybir.AxisListType.X, op=ALU.add)
                rsum = dsb.tile([1, Bl], F32, tag="rsum")
                nc.vector.reciprocal(rsum[:, :], ssum[:, :])
                rs = rsum[0:1, :]
                rsbc = bass.AP(tensor=rs.tensor, offset=rs.offset,
                               ap=[rs.ap[0], [0, S], rs.ap[1]])
                nc.vector.tensor_tensor(
                    out=es[0:1, 0:S * Bl].rearrange("p (s b) -> p s b", b=Bl),
                    in0=es[0:1, 0:S * Bl].rearrange("p (s b) -> p s b", b=Bl),
                    in1=rsbc, op=ALU.mult)
                # w transpose + block-diag mask (into psc corner)
                pwT = pscf
                for cq in range(NQ):
                    nc.tensor.transpose(out=pwT[:, cq:cq + 1],
                                        in_=es[0:1, 128 * cq:128 * (cq + 1)],
                                        identity=I128[:1, :1])
                wTm = dsb.tile([128, NQ, Bl], BF16, tag="wTm")
                for cq in range(NQ):
                    nc.vector.tensor_mul(wTm[:, cq, :],
                                         pwT[:, cq:cq + 1].to_broadcast([128, Bl]),
                                         mask4_s[:, :])
                # ctx-part of gates + ctx output
                for n in range(GN):
                    gsl = gates[:, 512 * n:512 * (n + 1)]
                    for cq in range(NQ):
                        nc.tensor.matmul(
                            out=gsl, lhsT=(wTm[:, cq, :]),
                            rhs=(encWcQ[:, cq, 512 * n:512 * (n + 1)]),
                            start=False, stop=(cq == NQ - 1))
                pcfull = dps.tile([128, A], F32, space="PSUM", tag="qctx")
                pctx = pcfull[0:Bl, 0:2 * H]
                for cq in range(NQ):
                    nc.tensor.matmul(out=pctx[:, 0:512], lhsT=(wTm[:, cq, :]),
                                     rhs=(ofQ[:, cq, :]),
                                     start=(cq == 0), stop=(cq == NQ - 1))
                for cq in range(NQ):
                    nc.tensor.matmul(out=pctx[:, 512:1024], lhsT=(wTm[:, cq, :]),
                                     rhs=(obQ[:, cq, :]),
                                     start=(cq == 0), stop=(cq == NQ - 1))
                ctst = dsb.tile([Bl, 2 * H], BF16, tag="ctst")
                nc.scalar.copy(ctst[:, :], pctx[:, :])
                nc.gpsimd.dma_start(
                    out=ctx_d[ds(t_expr, 1)].flatten_outer_dims(), in_=ctst[:, :])
                cell_update(gates, h, c, dcellp, "d")
                hbf = dcellp.tile([Bl, H], BF16, tag="dhbf")
                nc.vector.tensor_copy(hbf[:, :], h[:, :])
                nc.gpsimd.dma_start(
                    out=dec_d[ds(t_expr, 1)].flatten_outer_dims(), in_=hbf[:, :])

            with tc.For_i(0, T, unroll) as t0:
                for u_ in range(unroll):
                    dec_step(t0 + u_)

        scopeB_cm.__exit__(None, None, None)

        # ================= PHASE 5: classifier =================
        with tc.tile_pool(name="ph5", bufs=1) as ph5, \
             tc.tile_pool(name="ph5st", bufs=2) as ph5st, \
             tc.tile_pool(name="c1wp", bufs=1) as c1wp, \
             tc.tile_pool(name="ph5ps", bufs=1, space="PSUM") as ph5ps, \
             tc.tile_pool(name="c2psp", bufs=2, space="PSUM") as c2psp, \
             tc.tile_pool(name="ph5o", bufs=2) as ph5o:
            ctxQ = ph5.tile([128, NQT, 2 * H], BF16)
            decQ = ph5.tile([128, NQT, H], BF16)
            nc.vector.memset(ctxQ[:, :, :], 0.0)
            nc.vector.memset(decQ[:, :, :], 0.0)
            for q in range(NQT):
                rows = min(32, T - 32 * q) * Bl
                nc.gpsimd.dma_start(
                    out=ctxQ[:rows, q, :],
                    in_=ctx_d[32 * q:32 * q + rows // Bl].flatten_outer_dims())
                nc.gpsimd.dma_start(
                    out=decQ[:rows, q, :],
                    in_=dec_d[32 * q:32 * q + rows // Bl].flatten_outer_dims())
            ctxT = ph5.tile([128, H2K, 128 * NQT], BF16)
            decT = ph5.tile([128, HK, 128 * NQT], BF16)
            for q in range(NQT):
                pT = ph5ps.tile([128, H2K, 128], BF16, space="PSUM", tag="tps")
                for kc in range(H2K):
                    nc.tensor.transpose(out=pT[:, kc, :],
                                        in_=ctxQ[:, q, 128 * kc:128 * (kc + 1)],
                                        identity=I128b[:, :])
                nc.vector.tensor_copy(ctxT[:, :, 128 * q:128 * (q + 1)], pT[:, :, :])
                pT2 = ph5ps.tile([128, HK, 128], BF16, space="PSUM", tag="tps2")
                for kc in range(HK):
                    nc.tensor.transpose(out=pT2[:, kc, :],
                                        in_=decQ[:, q, 128 * kc:128 * (kc + 1)],
                                        identity=I128b[:, :])
                nc.vector.tensor_copy(decT[:, :, 128 * q:128 * (q + 1)], pT2[:, :, :])

            cls1b_s = ph5.tile([1, 1024], BF16)
            nc.gpsimd.dma_start(out=cls1b_s[:, :], in_=cls1b[:, :])
            h1Q = ph5.tile([128, NQT, 1024], BF16)
            lhs_chunks = ([(tembT, kc) for kc in range(EK)]
                          + [(ctxT, kc) for kc in range(H2K)]
                          + [(decT, kc) for kc in range(HK)])
            for n in range(2):
                c1w = c1wp.tile([128, G // 128, 512], BF16, tag="c1w")
                nc.gpsimd.dma_start(out=c1w[:, :, :],
                    in_=cls1WT[:, 512 * n:512 * (n + 1)].rearrange(
                        "(k p) n2 -> p k n2", p=128))
                for Tt in range(NQT):
                    ps = ph5ps.tile([128, 512], F32, space="PSUM", tag="c1ps")
                    for kg, (lt, kc) in enumerate(lhs_chunks):
                        nc.tensor.matmul(
                            out=ps[:, :],
                            lhsT=(lt[:, kc, 128 * Tt:128 * (Tt + 1)]),
                            rhs=(c1w[:, kg, :]),
                            start=(kg == 0), stop=False)
                    nc.tensor.matmul(out=ps[:, :], lhsT=(ones[:1, :128]),
                                     rhs=(cls1b_s[:, 512 * n:512 * (n + 1)]),
                                     start=False, stop=True)
                    nc.scalar.activation(out=h1Q[:, Tt, 512 * n:512 * (n + 1)],
                                         in_=ps[:, :], func=AF.Relu)
            h1T = ph5.tile([128, 8, 128 * NQT], BF16)
            for q in range(NQT):
                pT = ph5ps.tile([128, 8, 128], BF16, space="PSUM", tag="tps3")
                for kc in range(8):
                    nc.tensor.transpose(out=pT[:, kc, :],
                                        in_=h1Q[:, q, 128 * kc:128 * (kc + 1)],
                                        identity=I128b[:, :])
                nc.vector.tensor_copy(h1T[:, :, 128 * q:128 * (q + 1)], pT[:, :, :])

            def cls2_chunk(nv, nw):
                Wc = ph5st.tile([128, 8, 512], BF16, tag="c2w")
                nc.gpsimd.dma_start(
                    out=Wc[:, :, :nw],
                    in_=cls2WT[:, ds(nv, nw)].rearrange("(k p) n -> p k n", p=128))
                bc_ = ph5st.tile([1, 512], BF16, tag="c2b")
                nc.gpsimd.dma_start(out=bc_[:, :nw], in_=cls2b[0:1, ds(nv, nw)])
                for Tt in range(NQT):
                    trows = min(32, T - 32 * Tt)
                    ps = c2psp.tile([128, 512], F32, space="PSUM", tag="c2ps")
                    for kc in range(8):
                        nc.tensor.matmul(
                            out=ps[:, :nw],
                            lhsT=(h1T[:, kc, 128 * Tt:128 * (Tt + 1)]),
                            rhs=(Wc[:, kc, :nw]), start=(kc == 0), stop=False)
                    nc.tensor.matmul(out=ps[:, :nw], lhsT=(ones[:1, :128]),
                                     rhs=(bc_[:, :nw]), start=False, stop=True)
                    ost = ph5o.tile([128, 512], F32, tag="ost")
                    nc.vector.tensor_copy(ost[:, :nw], ps[:, :nw])
                    nc.gpsimd.dma_start(
                        out=logits[:, :, :].flatten_outer_dims()[
                            128 * Tt:128 * Tt + trows * Bl, ds(nv, nw)],
                        in_=ost[:trows * Bl, :nw])

            nfull = V // 512
            nd = (nfull // 4) * 4
            if nd > 0:
                with tc.For_i(0, 512 * nd, 2048) as nv0:
                    for uu in range(4):
                        cls2_chunk(nv0 + 512 * uu, 512)
            for start in range(512 * nd, V, 512):
                cls2_chunk(start, min(512, V - start))

        _stack.close()
    return nc


def prep_shared(p, V):
    """Core-independent input prep. p: dict of full-model params."""
    d = {}
    d["src_emb"] = np.ascontiguousarray(p["src_emb"], np.float32)
    d["tgt_emb"] = np.ascontiguousarray(p["tgt_emb"], np.float32)
    d["WihfT"] = np.ascontiguousarray(p["enc_Wih_f"].T).astype(BF)
    d["WihbT"] = np.ascontiguousarray(p["enc_Wih_b"].T).astype(BF)
    d["WhhfT"] = np.ascontiguousarray(p["enc_Whh_f"].T).astype(BF)
    d["WhhbT"] = np.ascontiguousarray(p["enc_Whh_b"].T).astype(BF)
    d["biasf"] = (p["enc_bih_f"] + p["enc_bhh_f"]).astype(BF)[None, :]
    d["biasb"] = (p["enc_bih_b"] + p["enc_bhh_b"]).astype(BF)[None, :]
    d["WiheT"] = np.ascontiguousarray(p["dec_Wih"][:, :E].T).astype(BF)
    d["WihcT"] = np.ascontiguousarray(p["dec_Wih"][:, E:].T).astype(BF)
    d["biasd"] = (p["dec_bih"] + p["dec_bhh"]).astype(BF)[None, :]
    d["WqWhhT"] = np.ascontiguousarray(
        np.concatenate([p["att1_W"][:, 2 * H:].T, p["dec_Whh"].T],
                       axis=1)).astype(BF)
    d["A1eT"] = np.ascontiguousarray(p["att1_W"][:, :2 * H].T).astype(BF)
    d["att1b"] = p["att1_b"].astype(BF)[None, :]
    d["a2T"] = np.ascontiguousarray(
        p["att2_W"][0].reshape(A // 128, 128).T, np.float32)
    d["cls1WT"] = np.ascontiguousarray(p["cls1_W"].T).astype(BF)
    d["cls1b"] = p["cls1_b"].astype(BF)[None, :]
    d["cls2WT"] = np.ascontiguousarray(p["cls2_W"].T).astype(BF)
    d["cls2b"] = p["cls2_b"].astype(BF)[None, :]
    mask4 = np.zeros((128, Bl), np.float32)
    for pp in range(128):
        mask4[pp, pp % Bl] = 1.0
    d["mask4"] = mask4
    return d


def idx_tile(tok, S):
    """tok: [Bl, S] int array -> [128, NQ] int32, token p=4*s'+b."""
    NQ = (S + 31) // 32
    out = np.zeros((128, NQ), np.int32)
    for q in range(NQ):
        for pp in range(128):
            b, sp = pp % Bl, pp // Bl
            s = 32 * q + sp
            if s < S:
                out[pp, q] = tok[b, s]
    return out


def prep_core(shared, source_data, target_data, core, S, T):
    d = dict(shared)
    d["idx_src"] = idx_tile(source_data[4 * core:4 * core + 4], S)
    d["idx_tgt"] = idx_tile(target_data[4 * core:4 * core + 4], T)
    return d


def np_reference(src, tgt, p):
    """Port of reference.py for batch rows in src/tgt [B, S]."""
    def sig(x): return 1.0 / (1.0 + np.exp(-x))

    def lstm_step(x, h, c, Wih, Whh, bih, bhh):
        g = x @ Wih.T + h @ Whh.T + (bih + bhh)
        i, f, gg, o = np.split(g, 4, axis=-1)
        c = sig(f) * c + sig(i) * np.tanh(gg)
        h = sig(o) * np.tanh(c)
        return h, c

    B, S = src.shape
    T = tgt.shape[1]
    x = p["src_emb"][src].transpose(1, 0, 2).astype(np.float32)
    z = np.zeros((B, H), np.float32)
    hf, cf = z, z
    of = []
    for s in range(S):
        hf, cf = lstm_step(x[s], hf, cf, p["enc_Wih_f"], p["enc_Whh_f"],
                           p["enc_bih_f"], p["enc_bhh_f"])
        of.append(hf)
    hb, cb = z, z
    ob = []
    for s in range(S):
        hb, cb = lstm_step(x[S - 1 - s], hb, cb, p["enc_Wih_b"], p["enc_Whh_b"],
                           p["enc_bih_b"], p["enc_bhh_b"])
        ob.append(hb)
    of = np.stack(of); ob = np.stack(ob)
    enc = np.concatenate([of, ob[::-1]], -1).transpose(1, 0, 2)  # [B,S,2H]
    temb = p["tgt_emb"][tgt].astype(np.float32)                  # [B,T,E]
    h, c = hf, cf
    ctxs, decs = [], []
    for t in range(T):
        prev = np.broadcast_to(h[:, None, :], (B, S, H))
        ain = np.concatenate([enc, prev], -1)
        hid = np.maximum(ain @ p["att1_W"].T + p["att1_b"], 0.0)
        sc = hid @ p["att2_W"].T + p["att2_b"]
        w = np.exp(sc - sc.max(axis=1, keepdims=True))
        w = w / w.sum(axis=1, keepdims=True)
        ctx = (w * enc).sum(axis=1)
        h, c = lstm_step(np.concatenate([temb[:, t], ctx], -1), h, c,
                         p["dec_Wih"], p["dec_Whh"], p["dec_bih"], p["dec_bhh"])
        ctxs.append(ctx); decs.append(h)
    ctxs = np.stack(ctxs, 1); decs = np.stack(decs, 1)
    ci = np.concatenate([temb, ctxs, decs], -1)
    h1 = np.maximum(ci @ p["cls1_W"].T + p["cls1_b"], 0.0)
    return h1 @ p["cls2_W"].T + p["cls2_b"]

# ===================== host-side entry point =====================
_CACHE = {}


def _get_nc():
    if "nc" not in _CACHE:
        nc = build_nc(S=100, T=100, V=32000, num_devices=8, unroll=4)
        nc.compile()
        _CACHE["nc"] = nc
    return _CACHE["nc"]


def kernel(trace=False, **inputs):
    import os
    S = T = 100
    V = 32000
    B = 32
    from concourse.bass_utils import run_bass_kernel_spmd
    nc = _get_nc()
    shared = prep_shared(inputs, V)
    src = np.asarray(inputs["source_data"])
    tgt = np.asarray(inputs["target_data"])
    in_maps = [prep_core(shared, src, tgt, c, S, T) for c in range(8)]
    res = run_bass_kernel_spmd(nc, in_maps, core_ids=list(range(8)),
                               trace=trace)
    out = np.empty((B, T, V), np.float32)
    for c in range(8):
        lg = np.asarray(res.results[c]["logits"]).reshape(T, Bl, V)
        out[4 * c:4 * c + 4] = lg.transpose(1, 0, 2)
    if trace:
        _CACHE["exec_time_ns"] = res.exec_time_ns
        _CACHE["profile"] = res
    return out



# revision 21
# speedup vs baseline: 1.7351x; 1.7351x over previous
# Self-contained TRN2 Bass kernel for nn_Attention_NMT (B=32,S=T=100,H=E=512,V=32000).
# SPMD over 8 NeuronCores, batch-parallel (4 batch rows per core).
#
# v1 design:
#  - sigmoid(x) = 0.5*(1+tanh(x/2)); the 0.5 scales are folded into weights
#    host-side and states are stored as S=2c, H=2h.  All scalar-engine funcs
#    (tanh/exp/relu/copy) live in one activation table -> no table reloads.
#  - LSTM cells run transposed: gates^T in PSUM [128 gate-feat, 16 chunks, B]
#    so cell elementwise ops use all 128 lanes and h is produced directly in
#    the [feat, b] form the gate/attention matmuls consume.
#  - Gate GEMMs are weight-stationary; x@Wih+bias precomputed transposed and
#    preloaded into PSUM via identity matmul.
#  - Attention uses exact 400 tokens; per-step context outputs are deferred:
#    only the 128x16 attention-weight tile is stored per step and all T
#    context vectors come from one batched GEMM after the decoder loop.
#  - Dummy dependent matmuls keep the PE HAM clock warm across serial spans.
import sys
for _p in ("/opt/trn_rl_repo",):
    if _p not in sys.path:
        sys.path.insert(0, _p)
import numpy as np
import ml_dtypes
BF = ml_dtypes.bfloat16
import concourse.bass as bass
import concourse.bacc as bacc
import concourse.tile as tile
from concourse import mybir
from concourse.bass import ds
from concourse.masks import make_identity

F32 = mybir.dt.float32
BF16 = mybir.dt.bfloat16
I32 = mybir.dt.int32
AF = mybir.ActivationFunctionType
ALU = mybir.AluOpType

E = 512; H = 512; G = 2048; A = 1536; Bl = 4
EK = E // 128; HK = H // 128; H2K = 2 * H // 128; AJ = A // 128
GC = G // 128
GN = G // 512


def build_nc(S=100, T=100, V=32000, num_devices=8, unroll=4):
    NQ = (S + 31) // 32
    NQT = (T + 31) // 32
    NT = S * Bl              # 400 real source tokens

    nc = bacc.Bacc("TRN2", target_bir_lowering=False, debug=False,
                   num_devices=num_devices)

    def din(name, shape, dt=F32):
        return nc.dram_tensor(name, shape, dt, kind="ExternalInput")

    src_emb = din("src_emb", [V, E]); tgt_emb = din("tgt_emb", [V, E])
    idx_src = din("idx_src", [128, NQ], I32)
    idx_tgt = din("idx_tgt", [128, NQT], I32)
    WihfT = din("WihfT", [E, G], BF16); WihbT = din("WihbT", [E, G], BF16)
    WiheT = din("WiheT", [E, G], BF16)
    WhhfT = din("WhhfT", [H, G], BF16); WhhbT = din("WhhbT", [H, G], BF16)
    WhhdT = din("WhhdT", [H, G], BF16)
    WqT = din("WqT", [H, A], BF16)
    WihcT = din("WihcT", [2 * H, G], BF16)
    A1eT = din("A1eT", [2 * H, A], BF16)
    a2T_in = din("a2T", [128, AJ])
    biasf = din("biasf", [1, G], BF16); biasb = din("biasb", [1, G], BF16)
    biasd = din("biasd", [1, G], BF16); att1b = din("att1b", [1, A], BF16)
    cls1WT = din("cls1WT", [G, 1024], BF16); cls1b = din("cls1b", [1, 1024], BF16)
    cls2WT = din("cls2WT", [1024, V], BF16); cls2b = din("cls2b", [1, V], BF16)
    mask4 = din("mask4", [128, Bl])

    logits = nc.dram_tensor("logits", [T, Bl, V], F32, kind="ExternalOutput")

    # DRAM scratch, transposed per-step layouts [step, 128 feat, chunk*b]
    Xf_d = nc.dram_tensor("Xf_d", [S, 128, GC * Bl], BF16)
    Xb_d = nc.dram_tensor("Xb_d", [S, 128, GC * Bl], BF16)
    Xd_d = nc.dram_tensor("Xd_d", [T, 128, GC * Bl], BF16)
    of_d = nc.dram_tensor("of_d", [S, 128, HK * Bl], BF16)
    ob_d = nc.dram_tensor("ob_d", [S, 128, HK * Bl], BF16)
    dec_d = nc.dram_tensor("dec_d", [T, 128, HK * Bl], BF16)
    w_d = nc.dram_tensor("w_d", [T, 128, NQ * Bl], BF16)

    with tile.TileContext(nc) as tc:
        from contextlib import ExitStack
        _stack = ExitStack()
        persist = _stack.enter_context(tc.tile_pool(name="persist", bufs=1))

        # ---- constants ----
        I128 = persist.tile([128, 128], F32)
        make_identity(nc, I128[:, :])
        I128b = persist.tile([128, 128], BF16)
        nc.vector.tensor_copy(I128b[:, :], I128[:, :])
        ones = persist.tile([1, 512], BF16)
        nc.vector.memset(ones[:, :], 1.0)
        mask4_s = persist.tile([128, Bl], F32)
        nc.gpsimd.dma_start(out=mask4_s[:, :], in_=mask4[:, :])
        a2Tb = persist.tile([128, AJ], BF16)
        a2Tf = persist.tile([128, AJ], F32)
        nc.gpsimd.dma_start(out=a2Tf[:, :], in_=a2T_in[:, :])
        nc.vector.tensor_copy(a2Tb[:, :], a2Tf[:, :])
        att1b_s = persist.tile([1, A], BF16)
        nc.gpsimd.dma_start(out=att1b_s[:, :], in_=att1b[:, :])

        tembT = persist.tile([128, EK, 128 * NQT], BF16)
        es = persist.tile([1, 512], F32)
        nc.vector.memset(es[:, :], 0.0)
        TP = 32 * NQT
        ctxT_sb = persist.tile([128, H2K, TP, Bl], BF16)
        # LSTM states, transposed [128 feat, HK, Bl]; stored as 2c / 2h
        h_f = persist.tile([128, HK, Bl], F32); c_f = persist.tile([128, HK, Bl], F32)
        h_b = persist.tile([128, HK, Bl], F32); c_b = persist.tile([128, HK, Bl], F32)
        for t_ in (h_f, c_f, h_b, c_b):
            nc.vector.memset(t_[:, :, :], 0.0)

        # ================= PHASE 0: embeddings + X^T GEMMs =================
        with tc.tile_pool(name="ph0", bufs=1) as ph0, \
             tc.tile_pool(name="ph0ps", bufs=2, space="PSUM") as ph0ps, \
             tc.tile_pool(name="ph0st", bufs=2) as ph0st:
            idxs = ph0.tile([128, NQ], I32)
            nc.gpsimd.dma_start(out=idxs[:, :], in_=idx_src[:, :])
            idxt = ph0.tile([128, NQT], I32)
            nc.gpsimd.dma_start(out=idxt[:, :], in_=idx_tgt[:, :])
            xQ = ph0.tile([128, NQ, E], F32)
            tembQ = ph0.tile([128, NQT, E], F32)
            for q in range(NQ):
                nc.gpsimd.indirect_dma_start(
                    out=xQ[:, q, :], out_offset=None, in_=src_emb[:, :],
                    in_offset=bass.IndirectOffsetOnAxis(ap=idxs[:, q:q + 1], axis=0))
            for q in range(NQT):
                nc.gpsimd.indirect_dma_start(
                    out=tembQ[:, q, :], out_offset=None, in_=tgt_emb[:, :],
                    in_offset=bass.IndirectOffsetOnAxis(ap=idxt[:, q:q + 1], axis=0))

            xT = ph0.tile([128, EK, 128 * NQ], BF16)
            for q in range(NQ):
                pT = ph0ps.tile([128, EK, 128], F32, space="PSUM")
                for kc in range(EK):
                    nc.tensor.transpose(out=pT[:, kc, :],
                                        in_=xQ[:, q, 128 * kc:128 * (kc + 1)],
                                        identity=I128[:, :])
                nc.vector.tensor_copy(xT[:, :, 128 * q:128 * (q + 1)], pT[:, :, :])
            for q in range(NQT):
                pT = ph0ps.tile([128, EK, 128], F32, space="PSUM")
                for kc in range(EK):
                    nc.tensor.transpose(out=pT[:, kc, :],
                                        in_=tembQ[:, q, 128 * kc:128 * (kc + 1)],
                                        identity=I128[:, :])
                nc.vector.tensor_copy(tembT[:, :, 128 * q:128 * (q + 1)], pT[:, :, :])

            # X^T GEMMs: out[gfeat, token] -> DRAM [s, 128, gc*b]
            def x_gemm(wT_dram, bias_dram, lhsT_tile, S_, out_dram, tag):
                Ws = ph0st.tile([128, EK, G], BF16, tag="wstream")
                nc.gpsimd.dma_start(
                    out=Ws[:, :, :],
                    in_=wT_dram[:, :].rearrange("(k p) g -> p k g", p=128))
                bia = ph0st.tile([1, G], BF16, tag="bias")
                nc.gpsimd.dma_start(out=bia[:, :], in_=bias_dram[:, :])
                stage = ph0.tile([128, S_, GC, Bl], BF16, tag="xstage" + tag)
                NT_ = S_ * Bl
                for gc in range(GC):
                    ps = ph0ps.tile([128, 512], F32, space="PSUM", tag="xps")
                    for kc in range(EK):
                        nc.tensor.matmul(
                            out=ps[:, :NT_],
                            lhsT=(Ws[:, kc, 128 * gc:128 * (gc + 1)]),
                            rhs=(lhsT_tile[:, kc, :NT_]),
                            start=(kc == 0), stop=False)
                    nc.tensor.matmul(
                        out=ps[:, :NT_],
                        lhsT=(bia[:1, 128 * gc:128 * (gc + 1)]),
                        rhs=(ones[:1, :NT_]), start=False, stop=True)
                    nc.vector.tensor_copy(
                        stage[:, :, gc, :],
                        ps[:, :NT_].rearrange("p (s b) -> p s b", b=Bl))
                nc.gpsimd.dma_start(
                    out=out_dram[:, :, :].rearrange("s p c -> p s c"),
                    in_=stage[:, :, :, :])

            x_gemm(WihfT, biasf, xT, S, Xf_d, "f")
            x_gemm(WihbT, biasb, xT, S, Xb_d, "b")
            x_gemm(WiheT, biasd, tembT, T, Xd_d, "d")

        # ================= PHASE 1: encoder =================
        with tc.tile_pool(name="ph1w", bufs=1) as ph1w, \
             tc.tile_pool(name="xst", bufs=2) as xstp, \
             tc.tile_pool(name="cell", bufs=2) as cellp, \
             tc.tile_pool(name="encg", bufs=2, space="PSUM") as encg:
            Whhf_s = ph1w.tile([128, HK, GC, 128], BF16)
            nc.gpsimd.dma_start(
                out=Whhf_s[:, :, :, :],
                in_=WhhfT[:, :].rearrange("(k p) (c m) -> p k c m", p=128, m=128))
            Whhb_s = ph1w.tile([128, HK, GC, 128], BF16)
            nc.gpsimd.dma_start(
                out=Whhb_s[:, :, :, :],
                in_=WhhbT[:, :].rearrange("(k p) (c m) -> p k c m", p=128, m=128))

            def cell_update(gps, h, c, pool, tagp):
                # gates order [f i o g]; returns nothing, updates h,c (2h/2c)
                tifo = pool.tile([128, 12, Bl], F32, tag="tifo" + tagp)
                nc.scalar.activation(out=tifo[:, :, :], in_=gps[:, 0:12, :],
                                     func=AF.Tanh)
                tg = pool.tile([128, HK, Bl], F32, tag="tg" + tagp)
                nc.scalar.activation(out=tg[:, :, :], in_=gps[:, 12:16, :],
                                     func=AF.Tanh)
                u1 = pool.tile([128, HK, Bl], F32, tag="u1" + tagp)
                nc.vector.scalar_tensor_tensor(
                    out=u1[:, :, :], in0=tifo[:, 0:4, :], scalar=1.0,
                    in1=c[:, :, :], op0=ALU.add, op1=ALU.mult)
                u2 = pool.tile([128, HK, Bl], F32, tag="u2" + tagp)
                nc.vector.scalar_tensor_tensor(
                    out=u2[:, :, :], in0=tifo[:, 4:8, :], scalar=1.0,
                    in1=tg[:, :, :], op0=ALU.add, op1=ALU.mult)
                nc.vector.scalar_tensor_tensor(
                    out=c[:, :, :], in0=u1[:, :, :], scalar=0.5,
                    in1=u2[:, :, :], op0=ALU.mult, op1=ALU.add)
                tc_t = pool.tile([128, HK, Bl], F32, tag="tc" + tagp)
                nc.scalar.activation(out=tc_t[:, :, :], in_=c[:, :, :],
                                     func=AF.Tanh, scale=0.5)
                nc.vector.scalar_tensor_tensor(
                    out=h[:, :, :], in0=tifo[:, 8:12, :], scalar=1.0,
                    in1=tc_t[:, :, :], op0=ALU.add, op1=ALU.mult)

            def lstm_stepT(h, c, Whh_s, X_d, s_expr, store_d, tagp):
                xst = xstp.tile([128, GC, Bl], BF16, tag="xst" + tagp)
                nc.gpsimd.dma_start(
                    out=xst[:, :, :],
                    in_=X_d[ds(s_expr, 1)].flatten_outer_dims())
                hbf = cellp.tile([128, HK, Bl], BF16, tag="hbf" + tagp)
                nc.vector.tensor_copy(hbf[:, :, :], h[:, :, :])
                gps = encg.tile([128, GC, Bl], F32, space="PSUM", tag="g" + tagp)
                nc.tensor.matmul(out=gps[:, :, :], lhsT=I128b[:, :],
                                 rhs=xst[:, :, :], start=True, stop=False)
                for gc in range(GC):
                    for kc in range(HK):
                        nc.tensor.matmul(
                            out=gps[:, gc, :],
                            lhsT=Whh_s[:, kc, gc, :],
                            rhs=hbf[:, kc, :],
                            start=False, stop=(kc == HK - 1))
                cell_update(gps, h, c, cellp, tagp)
                ho = cellp.tile([128, HK, Bl], BF16, tag="ho" + tagp)
                nc.vector.tensor_copy(ho[:, :, :], h[:, :, :])
                nc.gpsimd.dma_start(
                    out=store_d[ds(s_expr, 1)].flatten_outer_dims(),
                    in_=ho[:, :, :])

            with tc.For_i(0, S, unroll) as i0:
                for u_ in range(unroll):
                    lstm_stepT(h_f, c_f, Whhf_s, Xf_d, i0 + u_, of_d, "f")
                    lstm_stepT(h_b, c_b, Whhb_s, Xb_d, (S - 1 - u_) - i0, ob_d, "b")

        # ===== PHASE 2: load enc outputs; build token-row forms =====
        scopeB_cm = tc.tile_pool(name="scopeB", bufs=1)
        scopeB = scopeB_cm.__enter__()
        ofT_sb = scopeB.tile([128, HK, S, Bl], BF16)   # [feat, kc, s, b]
        obT_sb = scopeB.tile([128, HK, S, Bl], BF16)
        ofQ = scopeB.tile([128, NQ, H], BF16)          # [token, q, feat]
        obQ = scopeB.tile([128, NQ, H], BF16)
        u_s = scopeB.tile([128, AJ, S, Bl], BF16)
        encWcQ = scopeB.tile([128, NQ, G], BF16)
        nc.vector.memset(encWcQ[:, :, :], 0.0)
        with tc.tile_pool(name="ph2ps", bufs=2, space="PSUM") as ph2ps:
            oraw = scopeB.tile([128, S, HK, Bl], BF16, tag="oraw")
            braw = scopeB.tile([128, S, HK, Bl], BF16, tag="braw")
            nc.gpsimd.dma_start(
                out=oraw[:, :, :, :],
                in_=of_d[:, :, :].rearrange("s p x -> p s x"))
            nc.gpsimd.dma_start(
                out=braw[:, :, :, :],
                in_=ob_d[:, :, :].rearrange("s p x -> p s x"))
            for kc in range(HK):
                nc.vector.tensor_copy(ofT_sb[:, kc, :, :], oraw[:, :, kc, :])
                nc.vector.tensor_copy(obT_sb[:, kc, :, :], braw[:, :, kc, :])
            nc.vector.memset(ofQ[:, :, :], 0.0)
            nc.vector.memset(obQ[:, :, :], 0.0)
            for src, dst in ((ofT_sb, ofQ), (obT_sb, obQ)):
                for q in range(NQ):
                    rows = min(32, S - 32 * q) * Bl
                    pT = ph2ps.tile([128, HK, 128], BF16, space="PSUM")
                    for kc in range(HK):
                        nc.tensor.transpose(
                            out=pT[:rows, kc, :],
                            in_=src[:, kc, 32 * q:32 * q + rows // Bl, :],
                            identity=I128b[:, :])
                    nc.vector.tensor_copy(dst[:rows, q, :], pT[:rows, :, :])

        # ============ PHASE 3: u GEMM + encWcQ GEMM ============
        with tc.tile_pool(name="ph3", bufs=1) as ph3, \
             tc.tile_pool(name="ph3st", bufs=3) as ph3st, \
             tc.tile_pool(name="ph3ps", bufs=2, space="PSUM") as ph3ps:
            A1e_s = ph3.tile([128, H2K, A], BF16)
            nc.gpsimd.dma_start(out=A1e_s[:, :, :],
                in_=A1eT[:, :].rearrange("(k p) a -> p k a", p=128))
            for m in range(AJ):
                ps = ph3ps.tile([128, 512], F32, space="PSUM", tag="ups")
                for kc in range(H2K):
                    src = ofT_sb if kc < HK else obT_sb
                    kk = kc if kc < HK else kc - HK
                    nc.tensor.matmul(
                        out=ps[:, :NT],
                        lhsT=(A1e_s[:, kc, 128 * m:128 * (m + 1)]),
                        rhs=src[:, kk, :, :],
                        start=(kc == 0), stop=False)
                nc.tensor.matmul(out=ps[:, :NT],
                                 lhsT=(att1b_s[:1, 128 * m:128 * (m + 1)]),
                                 rhs=(ones[:1, :NT]), start=False, stop=True)
                nc.vector.tensor_copy(
                    u_s[:, m, :, :],
                    ps[:, :NT].rearrange("p (s b) -> p s b", b=Bl))
            # encWcQ: token-stationary over feature-form lhsT
            for n in range(GN):
                Wc_n = ph3st.tile([128, H2K, 512], BF16, tag="wcn")
                nc.gpsimd.dma_start(out=Wc_n[:, :, :],
                    in_=WihcT[:, 512 * n:512 * (n + 1)].rearrange(
                        "(k p) g -> p k g", p=128))
                for Tt in range(NQ):
                    rows = min(32, S - 32 * Tt)
                    ps = ph3ps.tile([128, 512], F32, space="PSUM", tag="wcps")
                    for kc in range(H2K):
                        src = ofT_sb if kc < HK else obT_sb
                        kk = kc if kc < HK else kc - HK
                        nc.tensor.matmul(
                            out=ps[:rows * Bl, :],
                            lhsT=src[:, kk, 32 * Tt:32 * Tt + rows, :],
                            rhs=(Wc_n[:, kc, :]), start=(kc == 0),
                            stop=(kc == H2K - 1))
                    nc.scalar.copy(encWcQ[:rows * Bl, Tt, 512 * n:512 * (n + 1)],
                                   ps[:rows * Bl, :])

        # ================= PHASE 4: decoder =================
        h = h_f; c = c_f
        with tc.tile_pool(name="ph4w", bufs=1) as ph4w, \
             tc.tile_pool(name="dxst", bufs=2) as dxstp, \
             tc.tile_pool(name="dcell", bufs=2) as dcellp, \
             tc.tile_pool(name="dsb", bufs=2) as dsb, \
             tc.tile_pool(name="dgps", bufs=2, space="PSUM") as dgps, \
             tc.tile_pool(name="dqps", bufs=2, space="PSUM") as dqps, \
             tc.tile_pool(name="dsps", bufs=2, space="PSUM") as dsps:
            rl_s = ph4w.tile([128, AJ, S, Bl], BF16)
            Whhd_s = ph4w.tile([128, HK, GC, 128], BF16)
            nc.gpsimd.dma_start(
                out=Whhd_s[:, :, :, :],
                in_=WhhdT[:, :].rearrange("(k p) (c m) -> p k c m", p=128, m=128))
            Wq_s = ph4w.tile([128, HK, AJ, 128], BF16)
            nc.gpsimd.dma_start(
                out=Wq_s[:, :, :, :],
                in_=WqT[:, :].rearrange("(k p) (j m) -> p k j m", p=128, m=128))

            def dec_step(t_expr):
                xst = dxstp.tile([128, GC, Bl], BF16, tag="dx")
                nc.gpsimd.dma_start(
                    out=xst[:, :, :],
                    in_=Xd_d[ds(t_expr, 1)].flatten_outer_dims())
                hbf = dcellp.tile([128, HK, Bl], BF16, tag="dhbf")
                nc.vector.tensor_copy(hbf[:, :, :], h[:, :, :])
                # q^T, weight-stationary -> PSUM [128 afeat, AJ, Bl]
                qps = dqps.tile([128, AJ, Bl], F32, space="PSUM", tag="dq")
                for j in range(AJ):
                    for kc in range(HK):
                        nc.tensor.matmul(
                            out=qps[:, j, :],
                            lhsT=Wq_s[:, kc, j, :],
                            rhs=hbf[:, kc, :],
                            start=(kc == 0), stop=(kc == HK - 1))
                # gates: x preload + h part (ctx part accumulates later)
                gps = dgps.tile([128, GC, Bl], F32, space="PSUM", tag="dg")
                nc.tensor.matmul(out=gps[:, :, :], lhsT=I128b[:, :],
                                 rhs=xst[:, :, :], start=True, stop=False)
                for gc in range(GC):
                    for kc in range(HK):
                        nc.tensor.matmul(
                            out=gps[:, gc, :],
                            lhsT=Whhd_s[:, kc, gc, :],
                            rhs=hbf[:, kc, :],
                            start=False, stop=False)
                qTs = dsb.tile([128, AJ, Bl], BF16, tag="dqT")
                nc.vector.tensor_copy(qTs[:, :, :], qps[:, :, :])
                # attention rl = relu(u+q), chunk-interleaved with score MMs
                pscf = dsps.tile([128, 512], F32, space="PSUM", tag="psc")
                psc = pscf[0:1, :]
                for j in range(AJ):
                    qbc = bass.AP(tensor=qTs.tensor, offset=qTs.offset + j * Bl,
                                  ap=[qTs.ap[0], [0, S], [1, Bl]])
                    eng = nc.vector
                    eng.tensor_tensor(out=rl_s[:, j, :, :],
                                      in0=u_s[:, j, :, :], in1=qbc, op=ALU.add)
                    eng.tensor_scalar_max(rl_s[:, j, :, :], rl_s[:, j, :, :], 0.0)
                    nc.tensor.matmul(out=psc[0:1, 0:NT],
                                     lhsT=a2Tb[:, j:j + 1],
                                     rhs=rl_s[:, j, :, :],
                                     start=(j == 0), stop=(j == AJ - 1))
                # softmax
                nc.scalar.activation(out=es[0:1, 0:NT], in_=psc[0:1, 0:NT],
                                     func=AF.Exp)
                ssum = dsb.tile([1, Bl], F32, tag="ssum")
                nc.vector.tensor_reduce(
                    out=ssum[:, :],
                    in_=es[0:1, 0:NT].rearrange("p (s b) -> p b s", b=Bl),
                    axis=mybir.AxisListType.X, op=ALU.add)
                # dummy MM keeps PE warm during the softmax serial span
                nc.tensor.matmul(out=pscf[0:1, 448:512], lhsT=ssum[:1, 0:1],
                                 rhs=es[0:1, 0:64], start=True, stop=True)
                rsum = dsb.tile([1, Bl], F32, tag="rsum")
                nc.vector.reciprocal(rsum[:, :], ssum[:, :])
                rsbc = bass.AP(tensor=rsum.tensor, offset=rsum.offset,
                               ap=[rsum.ap[0], [0, S], [1, Bl]])
                nc.vector.tensor_tensor(
                    out=es[0:1, 0:NT].rearrange("p (s b) -> p s b", b=Bl),
                    in0=es[0:1, 0:NT].rearrange("p (s b) -> p s b", b=Bl),
                    in1=rsbc, op=ALU.mult)
                # w transpose (full 128 cols; es tail is zero) + mask
                pwT = pscf
                for cq in range(NQ):
                    nc.tensor.transpose(out=pwT[:, cq:cq + 1],
                                        in_=es[0:1, 128 * cq:128 * (cq + 1)],
                                        identity=I128[:1, :1])
                wTm = dsb.tile([128, NQ, Bl], BF16, tag="wTm")
                nc.vector.tensor_tensor(
                    out=wTm[:, :, :],
                    in0=bass.AP(tensor=pwT.tensor, offset=pwT.offset,
                                ap=[pwT.ap[0], [1, NQ], [0, Bl]]),
                    in1=bass.AP(tensor=mask4_s.tensor, offset=mask4_s.offset,
                                ap=[mask4_s.ap[0], [0, NQ], [1, Bl]]),
                    op=ALU.mult)
                nc.gpsimd.dma_start(
                    out=w_d[ds(t_expr, 1)].flatten_outer_dims(),
                    in_=wTm[:, :, :])
                # ctx part of gates
                for gc in range(GC):
                    for cq in range(NQ):
                        nc.tensor.matmul(
                            out=gps[:, gc, :],
                            lhsT=encWcQ[:, cq, 128 * gc:128 * (gc + 1)],
                            rhs=wTm[:, cq, :],
                            start=False, stop=(cq == NQ - 1))
                cell_update2(gps, h, c)
                ho = dcellp.tile([128, HK, Bl], BF16, tag="dho")
                nc.vector.tensor_copy(ho[:, :, :], h[:, :, :])
                nc.gpsimd.dma_start(
                    out=dec_d[ds(t_expr, 1)].flatten_outer_dims(),
                    in_=ho[:, :, :])

            def cell_update2(gps, h, c):
                tifo = dcellp.tile([128, 12, Bl], F32, tag="tifo")
                nc.scalar.activation(out=tifo[:, :, :], in_=gps[:, 0:12, :],
                                     func=AF.Tanh)
                tg = dcellp.tile([128, HK, Bl], F32, tag="tg")
                nc.scalar.activation(out=tg[:, :, :], in_=gps[:, 12:16, :],
                                     func=AF.Tanh)
                u1 = dcellp.tile([128, HK, Bl], F32, tag="u1")
                nc.vector.scalar_tensor_tensor(
                    out=u1[:, :, :], in0=tifo[:, 0:4, :], scalar=1.0,
                    in1=c[:, :, :], op0=ALU.add, op1=ALU.mult)
                u2 = dcellp.tile([128, HK, Bl], F32, tag="u2")
                nc.vector.scalar_tensor_tensor(
                    out=u2[:, :, :], in0=tifo[:, 4:8, :], scalar=1.0,
                    in1=tg[:, :, :], op0=ALU.add, op1=ALU.mult)
                # dummy MM keeps PE warm through the cell serial span
                dum = dsps.tile([128, 512], F32, space="PSUM", tag="dum")
                nc.tensor.matmul(out=dum[0:1, 0:16],
                                 lhsT=u1[0:1, 0:1, 0:1],
                                 rhs=u1[0:1, :, :], start=True, stop=True)
                nc.vector.scalar_tensor_tensor(
                    out=c[:, :, :], in0=u1[:, :, :], scalar=0.5,
                    in1=u2[:, :, :], op0=ALU.mult, op1=ALU.add)
                tc_t = dcellp.tile([128, HK, Bl], F32, tag="tc")
                nc.scalar.activation(out=tc_t[:, :, :], in_=c[:, :, :],
                                     func=AF.Tanh, scale=0.5)
                nc.vector.scalar_tensor_tensor(
                    out=h[:, :, :], in0=tifo[:, 8:12, :], scalar=1.0,
                    in1=tc_t[:, :, :], op0=ALU.add, op1=ALU.mult)

            with tc.For_i(0, T, unroll) as t0:
                for u_ in range(unroll):
                    dec_step(t0 + u_)

        # ============ PHASE 4b: deferred ctx^T GEMM ============
        with tc.tile_pool(name="ph4b", bufs=1) as ph4b, \
             tc.tile_pool(name="ph4bps", bufs=2, space="PSUM") as ph4bps:
            wAll = ph4b.tile([128, T, NQ, Bl], BF16)
            nc.gpsimd.dma_start(out=wAll[:, :, :, :],
                                in_=w_d[:, :, :].rearrange("t p x -> p t x"))
            for b in range(Bl):
                for half, encsrc in ((0, ofQ), (1, obQ)):
                    for fc in range(HK):
                        ps = ph4bps.tile([128, T], F32, space="PSUM", tag="cps")
                        for cq in range(NQ):
                            nc.tensor.matmul(
                                out=ps[:, :],
                                lhsT=encsrc[:, cq, 128 * fc:128 * (fc + 1)],
                                rhs=wAll[:, :, cq, b],
                                start=(cq == 0), stop=(cq == NQ - 1))
                        kc = half * HK + fc
                        if fc % 2 == 0:
                            nc.vector.tensor_copy(ctxT_sb[:, kc, 0:T, b], ps[:, :])
                        else:
                            nc.scalar.copy(ctxT_sb[:, kc, 0:T, b], ps[:, :])

        scopeB_cm.__exit__(None, None, None)

        # ================= PHASE 5: classifier =================
        with tc.tile_pool(name="ph5", bufs=1) as ph5, \
             tc.tile_pool(name="ph5st", bufs=2) as ph5st, \
             tc.tile_pool(name="c1wp", bufs=1) as c1wp, \
             tc.tile_pool(name="ph5ps", bufs=1, space="PSUM") as ph5ps, \
             tc.tile_pool(name="c2psp", bufs=2, space="PSUM") as c2psp, \
             tc.tile_pool(name="ph5o", bufs=2) as ph5o:
            decT_sb = ph5.tile([128, HK, 32 * NQT, Bl], BF16)
            draw = ph5.tile([128, T, HK, Bl], BF16)
            nc.gpsimd.dma_start(
                out=draw[:, :, :, :],
                in_=dec_d[:, :, :].rearrange("t p x -> p t x"))
            for kc in range(HK):
                nc.vector.tensor_copy(decT_sb[:, kc, 0:T, :], draw[:, :, kc, :])

            cls1b_s = ph5.tile([1, 1024], BF16)
            nc.gpsimd.dma_start(out=cls1b_s[:, :], in_=cls1b[:, :])
            h1Q = ph5.tile([128, NQT, 1024], BF16)
            for n in range(2):
                c1w = c1wp.tile([128, G // 128, 512], BF16, tag="c1w")
                nc.gpsimd.dma_start(out=c1w[:, :, :],
                    in_=cls1WT[:, 512 * n:512 * (n + 1)].rearrange(
                        "(k p) n2 -> p k n2", p=128))
                for Tt in range(NQT):
                    ps = ph5ps.tile([128, 512], F32, space="PSUM", tag="c1ps")
                    kg = 0
                    for kc in range(EK):
                        nc.tensor.matmul(
                            out=ps[:, :],
                            lhsT=(tembT[:, kc, 128 * Tt:128 * (Tt + 1)]),
                            rhs=(c1w[:, kg, :]), start=(kg == 0), stop=False)
                        kg += 1
                    for kc in range(H2K):
                        nc.tensor.matmul(
                            out=ps[:, :],
                            lhsT=ctxT_sb[:, kc, 32 * Tt:32 * (Tt + 1), :],
                            rhs=(c1w[:, kg, :]), start=False, stop=False)
                        kg += 1
                    for kc in range(HK):
                        nc.tensor.matmul(
                            out=ps[:, :],
                            lhsT=decT_sb[:, kc, 32 * Tt:32 * (Tt + 1), :],
                            rhs=(c1w[:, kg, :]), start=False, stop=False)
                        kg += 1
                    nc.tensor.matmul(out=ps[:, :], lhsT=(ones[:1, :128]),
                                     rhs=(cls1b_s[:, 512 * n:512 * (n + 1)]),
                                     start=False, stop=True)
                    nc.scalar.activation(out=h1Q[:, Tt, 512 * n:512 * (n + 1)],
                                         in_=ps[:, :], func=AF.Relu)
            h1T = ph5.tile([128, 8, 128 * NQT], BF16)
            for q in range(NQT):
                pT = ph5ps.tile([128, 8, 128], BF16, space="PSUM", tag="tps3")
                for kc in range(8):
                    nc.tensor.transpose(out=pT[:, kc, :],
                                        in_=h1Q[:, q, 128 * kc:128 * (kc + 1)],
                                        identity=I128b[:, :])
                nc.vector.tensor_copy(h1T[:, :, 128 * q:128 * (q + 1)], pT[:, :, :])

            def cls2_chunk(nv, nw):
                Wc = ph5st.tile([128, 8, 512], BF16, tag="c2w")
                nc.gpsimd.dma_start(
                    out=Wc[:, :, :nw],
                    in_=cls2WT[:, ds(nv, nw)].rearrange("(k p) n -> p k n", p=128))
                bc_ = ph5st.tile([1, 512], BF16, tag="c2b")
                nc.gpsimd.dma_start(out=bc_[:, :nw], in_=cls2b[0:1, ds(nv, nw)])
                for Tt in range(NQT):
                    trows = min(32, T - 32 * Tt)
                    ps = c2psp.tile([128, 512], F32, space="PSUM", tag="c2ps")
                    for kc in range(8):
                        nc.tensor.matmul(
                            out=ps[:, :nw],
                            lhsT=(h1T[:, kc, 128 * Tt:128 * (Tt + 1)]),
                            rhs=(Wc[:, kc, :nw]), start=(kc == 0), stop=False)
                    nc.tensor.matmul(out=ps[:, :nw], lhsT=(ones[:1, :128]),
                                     rhs=(bc_[:, :nw]), start=False, stop=True)
                    ost = ph5o.tile([128, 512], F32, tag="ost")
                    if (Tt % 2) == 0:
                        nc.vector.tensor_copy(ost[:, :nw], ps[:, :nw])
                    else:
                        nc.scalar.copy(ost[:, :nw], ps[:, :nw])
                    nc.gpsimd.dma_start(
                        out=logits[:, :, :].flatten_outer_dims()[
                            128 * Tt:128 * Tt + trows * Bl, ds(nv, nw)],
                        in_=ost[:trows * Bl, :nw])

            nfull = V // 512
            nd = (nfull // 4) * 4
            if nd > 0:
                with tc.For_i(0, 512 * nd, 2048) as nv0:
                    for uu in range(4):
                        cls2_chunk(nv0 + 512 * uu, 512)
            for start in range(512 * nd, V, 512):
                cls2_chunk(start, min(512, V - start))

        _stack.close()
    return nc


# ===================== host-side input prep =====================
def _reorder_rows(W4h):
    """torch gate order [i f g o] rows -> [f i o g], with f,i,o rows * 0.5."""
    i = W4h[0:512]; f = W4h[512:1024]; g = W4h[1024:1536]; o = W4h[1536:2048]
    return np.concatenate([0.5 * f, 0.5 * i, 0.5 * o, g], axis=0)


def prep_shared(p, V):
    d = {}
    d["src_emb"] = np.ascontiguousarray(p["src_emb"], np.float32)
    d["tgt_emb"] = np.ascontiguousarray(p["tgt_emb"], np.float32)

    Wihf = _reorder_rows(p["enc_Wih_f"])
    Wihb = _reorder_rows(p["enc_Wih_b"])
    Wihe = _reorder_rows(p["dec_Wih"][:, :E])
    # h inputs are stored as 2h -> extra 0.5 column scale
    Whhf = 0.5 * _reorder_rows(p["enc_Whh_f"])
    Whhb = 0.5 * _reorder_rows(p["enc_Whh_b"])
    Whhd = 0.5 * _reorder_rows(p["dec_Whh"])
    # ctx input: ctx built from 2h-stored enc outputs -> 0.5 column scale
    Wihc = 0.5 * _reorder_rows(p["dec_Wih"][:, E:])

    d["WihfT"] = np.ascontiguousarray(Wihf.T).astype(BF)
    d["WihbT"] = np.ascontiguousarray(Wihb.T).astype(BF)
    d["WiheT"] = np.ascontiguousarray(Wihe.T).astype(BF)
    d["WhhfT"] = np.ascontiguousarray(Whhf.T).astype(BF)
    d["WhhbT"] = np.ascontiguousarray(Whhb.T).astype(BF)
    d["WhhdT"] = np.ascontiguousarray(Whhd.T).astype(BF)
    d["WihcT"] = np.ascontiguousarray(Wihc.T).astype(BF)

    def bias_scale(b):
        i = b[0:512]; f = b[512:1024]; g = b[1024:1536]; o = b[1536:2048]
        return np.concatenate([0.5 * f, 0.5 * i, 0.5 * o, g], axis=0)

    d["biasf"] = bias_scale(p["enc_bih_f"] + p["enc_bhh_f"]).astype(BF)[None, :]
    d["biasb"] = bias_scale(p["enc_bih_b"] + p["enc_bhh_b"]).astype(BF)[None, :]
    d["biasd"] = bias_scale(p["dec_bih"] + p["dec_bhh"]).astype(BF)[None, :]

    d["WqT"] = np.ascontiguousarray((0.5 * p["att1_W"][:, 2 * H:]).T).astype(BF)
    d["A1eT"] = np.ascontiguousarray((0.5 * p["att1_W"][:, :2 * H]).T).astype(BF)
    d["att1b"] = p["att1_b"].astype(BF)[None, :]
    d["a2T"] = np.ascontiguousarray(
        p["att2_W"][0].reshape(A // 128, 128).T, np.float32)

    cls1 = p["cls1_W"].copy()
    cls1[:, E:E + 2 * H] *= 0.5      # ctx stored as 2*ctx
    cls1[:, E + 2 * H:] *= 0.5       # dec h stored as 2h
    d["cls1WT"] = np.ascontiguousarray(cls1.T).astype(BF)
    d["cls1b"] = p["cls1_b"].astype(BF)[None, :]
    d["cls2WT"] = np.ascontiguousarray(p["cls2_W"].T).astype(BF)
    d["cls2b"] = p["cls2_b"].astype(BF)[None, :]
    mask4 = np.zeros((128, Bl), np.float32)
    for pp in range(128):
        mask4[pp, pp % Bl] = 1.0
    d["mask4"] = mask4
    return d


def idx_tile(tok, S):
    NQ = (S + 31) // 32
    out = np.zeros((128, NQ), np.int32)
    for q in range(NQ):
        for pp in range(128):
            b, sp = pp % Bl, pp // Bl
            s = 32 * q + sp
            if s < S:
                out[pp, q] = tok[b, s]
    return out


def prep_core(shared, source_data, target_data, core, S, T):
    d = dict(shared)
    d["idx_src"] = idx_tile(source_data[4 * core:4 * core + 4], S)
    d["idx_tgt"] = idx_tile(target_data[4 * core:4 * core + 4], T)
    return d


# ===================== host-side entry point =====================
_CACHE = {}


def _get_nc():
    if "nc" not in _CACHE:
        nc = build_nc(S=100, T=100, V=32000, num_devices=8, unroll=4)
        nc.compile()
        _CACHE["nc"] = nc
    return _CACHE["nc"]


def kernel(trace=False, **inputs):
    S = T = 100
    V = 32000
    B = 32
    from concourse.bass_utils import run_bass_kernel_spmd
    nc = _get_nc()
    shared = prep_shared(inputs, V)
    src = np.asarray(inputs["source_data"])
    tgt = np.asarray(inputs["target_data"])
    in_maps = [prep_core(shared, src, tgt, c, S, T) for c in range(8)]
    res = run_bass_kernel_spmd(nc, in_maps, core_ids=list(range(8)),
                               trace=trace)
    out = np.empty((B, T, V), np.float32)
    for c in range(8):
        lg = np.asarray(res.results[c]["logits"]).reshape(T, Bl, V)
        out[4 * c:4 * c + 4] = lg.transpose(1, 0, 2)
    if trace:
        _CACHE["exec_time_ns"] = res.exec_time_ns
        _CACHE["profile"] = res
    return out
